# revision 5
# baseline (speedup 1.0000x reference)
"""ArgumentGCN (gnn_message_passing) Trainium2 kernel.

Data-parallel over batch: B=8 batches -> 8 NeuronCores, one batch per core.

Math per batch (N=2048 nodes, D=128 features, 2 iterations):
    graph[i,j] = mask[i]*mask[j]*(1-eye)*punct[i,j]
    nbr[i]     = max(sum_j graph[i,j], 1)
    per step:
      dw       = sigmoid(x @ w_nw.T + b_nw)            # [N]
      self     = x @ w_self.T + b_self                 # [N,D]
      info_p   = x @ w_punct.T                         # [N,D]
      agg[i]   = sum_j graph[i,j]*dw[j]*info_p[j] / nbr[i]
      x        = relu(self + agg)

Kernel formulation (per core):
    z[j,:]   = mask[j]*dw[j]*info_p[j,:]
    A^T      = sum_jb z[jb]-stationary @ punctT[jb]-moving   (PSUM accumulate)
    nbrS     = sum_jb mask[jb]-stationary @ punctT[jb]-moving
    x'       = relu(self + u[i]*A[i,:] - (r[i]*punct[i,i])*z[i,:])
               u = mask*r, r = 1/max(nbr,1)

punct is host-transposed so its row blocks land with the contraction index on
SBUF partitions; it is converted int32->bf16 once and kept in SBUF (8 MiB) so
iteration 2 reruns the big matmul without touching HBM again.

Engine policy: every tensor a matmul reads is last-written by the DVE (or is
part of the one DMA'd const tile), and each fresh PSUM tile is first touched
by a 1-element DVE memset ("breaker") so PSUM-slot-recycle deps never attach
to Matmult instructions — walrus only allows ONE sync wait on a matmul (it
lands on the lowered LDWEIGHTS) and mixed-engine deps would need two.
"""

import numpy as np
import ml_dtypes
from contextlib import ExitStack

import concourse.bass as bass
import concourse.tile as tile
from concourse import mybir
from concourse.bass_utils import run_bass_kernel_spmd

FP = mybir.dt.float32
BF = mybir.dt.bfloat16
I32 = mybir.dt.int32

N = 2048
D = 128
NB = N // 128  # 16 row/col blocks
NC = 8
CAT = 2 * D + 1  # 257 packed projection outputs: [self | punct | nw]
BO_W = CAT + 128  # bias_ones width: [bias_cat | ones(128)]


def _bs(i):  # block slice
    return slice(i * 128, (i + 1) * 128)


def build_nc(split_waits: bool = True) -> bass.Bass:
    nc = bass.Bass()

    punct_T = nc.declare_dram_parameter("punct_T", [N, N], I32, isOutput=False)
    node_T = nc.declare_dram_parameter("node_T", [D, N], FP, isOutput=False)
    mask_col = nc.declare_dram_parameter("mask_col", [128, NB], I32, isOutput=False)
    w_cat = nc.declare_dram_parameter("w_cat", [D, CAT], FP, isOutput=False)
    bias_ones = nc.declare_dram_parameter("bias_ones", [1, BO_W], FP, isOutput=False)
    ident_in = nc.declare_dram_parameter("ident", [128, 128], BF, isOutput=False)
    out_x = nc.declare_dram_parameter("out_x", [N, D], FP, isOutput=True)
    out_w = nc.declare_dram_parameter("out_w", [128, 2 * NB], FP, isOutput=True)

    with tile.TileContext(nc) as tc:
        with ExitStack() as ctx:
            _build(ctx, tc, nc, punct_T, node_T, mask_col, w_cat, bias_ones,
                   ident_in, out_x, out_w)
    if split_waits:
        _split_multi_waits(nc)
    return nc


def _split_multi_waits(nc: bass.Bass) -> int:
    """Walrus allows a single sync wait per ISA instruction (one EVENTS slot).
    Tile's sem-assignment can attach several; hoist the extras onto standalone
    EventSemaphore instructions on the same engine stream just before the
    instruction."""
    n = 0
    for fn in nc.m.functions:
        for blk in fn.blocks:
            out = []
            changed = False
            for inst in blk.instructions:
                si = inst.sync_info
                waits = list(si.on_wait) if si and si.on_wait else []
                if len(waits) > 1:
                    changed = True
                    for w in waits[:-1]:
                        ev = mybir.InstEventSemaphore(
                            name=f"antsplitw-{n}", ins=[], outs=[])
                        n += 1
                        ev.engine = inst.engine
                        ev.sync_info = mybir.SyncInfo(on_wait=[w], on_update=[])
                        out.append(ev)
                    inst.sync_info = mybir.SyncInfo(
                        on_wait=[waits[-1]],
                        on_update=list(si.on_update) if si.on_update else [])
                out.append(inst)
            if changed:
                blk.instructions = out
    return n


def _build(ctx, tc, nc, punct_T, node_T, mask_col, w_cat, bias_ones, ident_in,
           out_x, out_w):
    psum = ctx.enter_context(tc.tile_pool(name="psum", bufs=8, space="PSUM"))
    const = ctx.enter_context(tc.tile_pool(name="const", bufs=1))
    big = ctx.enter_context(tc.tile_pool(name="big", bufs=1))
    ptp = ctx.enter_context(tc.tile_pool(name="ptp", bufs=NB))
    pintp = ctx.enter_context(tc.tile_pool(name="pintp", bufs=3))
    scratch = ctx.enter_context(tc.tile_pool(name="scratch", bufs=4))

    def brk(ptile):
        # DVE single-element touch: absorbs PSUM-slot-recycle deps so the
        # following matmul carries at most one sync wait.
        nc.vector.memset(ptile[0:1, 0:1], 0.0)

    # ---- constants / small inputs ----
    ndT = big.tile([D, N], FP, tag="ndT", name="ndT")
    nc.sync.dma_start(ndT[:], node_T[:, :])
    wcatf = const.tile([D, CAT], FP, tag="wcatf", name="wcatf")
    nc.sync.dma_start(wcatf[:], w_cat[:, :])
    mask_i = const.tile([128, NB], I32, tag="mask_i", name="mask_i")
    nc.sync.dma_start(mask_i[:], mask_col[:, :])
    bo_sb = const.tile([1, BO_W], FP, tag="bo_sb", name="bo_sb")
    nc.sync.dma_start(bo_sb[:], bias_ones[:, :])
    ident_d = const.tile([128, 128], BF, tag="ident_d", name="ident_d")
    nc.sync.dma_start(ident_d[:], ident_in[:, :])

    ones_row = bo_sb[0:1, CAT : CAT + 128]
    ones_sc = bo_sb[0:1, CAT : CAT + 1]
    bias_row = bo_sb[0:1, 0:CAT]

    ident = const.tile([128, 128], BF, tag="ident", name="ident")
    nc.vector.tensor_copy(ident[:], ident_d[:])
    xT1 = big.tile([D, N], BF, tag="xT1", name="xT1")
    nc.vector.tensor_copy(xT1[:], ndT[:])
    wcatb = const.tile([D, CAT], BF, tag="wcatb", name="wcatb")
    nc.vector.tensor_copy(wcatb[:], wcatf[:])
    m_f = const.tile([128, NB], FP, tag="m_f", name="m_f")
    nc.vector.tensor_copy(m_f[:], mask_i[:])
    m_b = const.tile([128, NB], BF, tag="m_b", name="m_b")
    nc.vector.tensor_copy(m_b[:], m_f[:])

    dw_sb = const.tile([128, 2 * NB], FP, tag="dw_sb", name="dw_sb")
    Pdiag = const.tile([128, NB], FP, tag="Pdiag", name="Pdiag")

    z1 = big.tile([128, N], BF, tag="z1", name="z1")
    self1 = big.tile([128, N], FP, tag="self1", name="self1")

    # ---- projection for a step ----
    def proj(xT, z, self_sb, step):
        for nb in range(NB):
            pp = psum.tile([128, CAT], FP, tag="bank", name=f"pp{step}_{nb}")
            brk(pp)
            nc.tensor.matmul(pp[:], xT[:, _bs(nb)], wcatb[:], start=True, stop=False)
            nc.tensor.matmul(pp[:], ones_row, bias_row, start=False, stop=True)
            dwc = dw_sb[:, step * NB + nb : step * NB + nb + 1]
            nc.scalar.activation(dwc, pp[:, 2 * D : 2 * D + 1],
                                 mybir.ActivationFunctionType.Sigmoid)
            nc.scalar.copy(self_sb[:, _bs(nb)], pp[:, 0:D])
            mdw = scratch.tile([128, 1], FP, tag="mdw", bufs=3, name=f"mdw{step}_{nb}")
            nc.vector.tensor_mul(mdw[:], dwc, m_f[:, nb : nb + 1])
            nc.vector.tensor_scalar(z[:, _bs(nb)], pp[:, D : 2 * D], mdw[:], None,
                                    mybir.AluOpType.mult)

    proj(xT1, z1, self1, 0)

    # ---- stream punct^T, convert to bf16, accumulate step-1 matmuls ----
    A1T_ps = [psum.tile([128, 512], FP, tag="bank", name=f"a1t{c}") for c in range(4)]
    nbr_ps = [psum.tile([1, 512], FP, tag="bank", name=f"nbr{c}") for c in range(4)]
    for t in A1T_ps + nbr_ps:
        brk(t)
    PT = []
    for jb in range(NB):
        pint = pintp.tile([128, N], I32, tag="pint", name=f"pint{jb}")
        nc.sync.dma_start(pint[:], punct_T[_bs(jb), :])
        pt = ptp.tile([128, N], BF, tag="pt", name=f"pt{jb}")
        if jb % 2 == 0:
            nc.vector.tensor_copy(pt[:], pint[:])
        else:
            nc.scalar.copy(pt[:], pint[:])
        PT.append(pt)
        # diagonal of punct (block jb, jb) -> Pdiag[:, jb]
        dtmp = scratch.tile([128, 128], BF, tag="dtmp", bufs=2, name=f"dtmp{jb}")
        nc.vector.tensor_mul(dtmp[:], pt[:, _bs(jb)], ident[:])
        nc.vector.tensor_reduce(Pdiag[:, jb : jb + 1], dtmp[:],
                                mybir.AxisListType.X, mybir.AluOpType.add)
        for c in range(4):
            nc.tensor.matmul(A1T_ps[c][:], z1[:, _bs(jb)],
                             pt[:, c * 512 : (c + 1) * 512],
                             start=(jb == 0), stop=(jb == NB - 1))
        for c in range(4):
            nc.tensor.matmul(nbr_ps[c][:], m_b[:, jb : jb + 1],
                             pt[:, c * 512 : (c + 1) * 512],
                             start=(jb == 0), stop=(jb == NB - 1))

    # ---- step-1 wrap: neighbor counts + per-row factors, evacuate A^T ----
    nbrS_row = const.tile([1, N], FP, tag="nbrS_row", name="nbrS_row")
    for c in range(4):
        nc.scalar.copy(nbrS_row[:, c * 512 : (c + 1) * 512], nbr_ps[c][:])
    nbrT_ps = psum.tile([128, NB], FP, tag="bank", name="nbrT_ps")
    brk(nbrT_ps)
    for ib in range(NB):
        nc.tensor.matmul(nbrT_ps[:, ib : ib + 1], nbrS_row[:, _bs(ib)], ones_sc,
                         start=True, stop=True)
    nbr_all = const.tile([128, NB], FP, tag="nbr_all", name="nbr_all")
    nc.vector.tensor_copy(nbr_all[:], nbrT_ps[:])

    pm = const.tile([128, NB], FP, tag="pm", name="pm")
    nc.vector.tensor_mul(pm[:], Pdiag[:], m_f[:])
    nn_t = const.tile([128, NB], FP, tag="nn_t", name="nn_t")
    nc.vector.tensor_sub(nn_t[:], nbr_all[:], pm[:])
    nbr_num = const.tile([128, NB], FP, tag="nbr_num", name="nbr_num")
    nc.vector.tensor_mul(nbr_num[:], nn_t[:], m_f[:])
    nbr_f = const.tile([128, NB], FP, tag="nbr_f", name="nbr_f")
    nc.vector.tensor_scalar(nbr_f[:], nbr_num[:], 1.0, None, mybir.AluOpType.max)
    r_all = const.tile([128, NB], FP, tag="r_all", name="r_all")
    nc.vector.reciprocal(r_all[:], nbr_f[:])
    u_all = const.tile([128, NB], FP, tag="u_all", name="u_all")
    nc.vector.tensor_mul(u_all[:], r_all[:], m_f[:])
    rp = const.tile([128, NB], FP, tag="rp", name="rp")
    nc.vector.tensor_mul(rp[:], r_all[:], Pdiag[:])
    nv_all = const.tile([128, NB], FP, tag="nv_all", name="nv_all")
    nc.vector.tensor_scalar(nv_all[:], rp[:], -1.0, None, mybir.AluOpType.mult)

    A1T = big.tile([128, N], BF, tag="A1T", name="A1T")
    for c in range(4):
        cs = slice(c * 512, (c + 1) * 512)
        if c % 2 == 0:
            nc.vector.tensor_copy(A1T[:, cs], A1T_ps[c][:])
        else:
            nc.scalar.copy(A1T[:, cs], A1T_ps[c][:])

    # ---- per-block combine: relu(self + u*A - (r*Pdiag)*z) ----
    def combine(AT, z, self_sb, ib, out_ap, relu_engine):
        tb = psum.tile([128, 128], FP, tag="bank", name=f"tb_{AT.tensor.name}_{ib}")
        brk(tb)
        nc.tensor.matmul(tb[:], AT[:, _bs(ib)], ident[:], start=True, stop=True)
        pre1 = scratch.tile([128, 128], FP, tag="pre", bufs=4,
                            name=f"pre1_{AT.tensor.name}_{ib}")
        nc.vector.scalar_tensor_tensor(pre1[:], tb[:], u_all[:, ib : ib + 1],
                                       self_sb[:, _bs(ib)], mybir.AluOpType.mult,
                                       mybir.AluOpType.add)
        pre2 = scratch.tile([128, 128], FP, tag="pre", bufs=4,
                            name=f"pre2_{AT.tensor.name}_{ib}")
        nc.vector.scalar_tensor_tensor(pre2[:], z[:, _bs(ib)], nv_all[:, ib : ib + 1],
                                       pre1[:], mybir.AluOpType.mult,
                                       mybir.AluOpType.add)
        if relu_engine == "vector":
            nc.vector.tensor_relu(out_ap, pre2[:])
        else:
            nc.scalar.activation(out_ap, pre2[:], mybir.ActivationFunctionType.Relu)

    x2 = big.tile([128, N], BF, tag="x2", name="x2")
    xT2 = big.tile([D, N], BF, tag="xT2", name="xT2")
    for ib in range(NB):
        combine(A1T, z1, self1, ib, x2[:, _bs(ib)], "scalar")
        xt = psum.tile([128, 128], FP, tag="bank", name=f"xt{ib}")
        brk(xt)
        nc.tensor.matmul(xt[:], x2[:, _bs(ib)], ident[:], start=True, stop=True)
        if ib % 2 == 0:
            nc.vector.tensor_copy(xT2[:, _bs(ib)], xt[:])
        else:
            nc.scalar.copy(xT2[:, _bs(ib)], xt[:])

    # ---- step 2 ----
    z2 = big.tile([128, N], BF, tag="z2", name="z2")
    self2 = big.tile([128, N], FP, tag="self2", name="self2")
    proj(xT2, z2, self2, 1)

    A2T_ps = [psum.tile([128, 512], FP, tag="bank", name=f"a2t{c}") for c in range(4)]
    for t in A2T_ps:
        brk(t)
    for jb in range(NB):
        for c in range(4):
            nc.tensor.matmul(A2T_ps[c][:], z2[:, _bs(jb)],
                             PT[jb][:, c * 512 : (c + 1) * 512],
                             start=(jb == 0), stop=(jb == NB - 1))

    A2T = big.tile([128, N], BF, tag="A2T", name="A2T")
    for c in range(4):
        nc.vector.tensor_copy(A2T[:, c * 512 : (c + 1) * 512], A2T_ps[c][:])

    for ib in range(NB):
        x3 = scratch.tile([128, 128], FP, tag="x3", bufs=3, name=f"x3_{ib}")
        combine(A2T, z2, self2, ib, x3[:], "scalar")
        nc.sync.dma_start(out_x[_bs(ib), :], x3[:])

    nc.sync.dma_start(out_w[:, :], dw_sb[:])


_NC_CACHE = None


def _get_nc():
    global _NC_CACHE
    if _NC_CACHE is None:
        _NC_CACHE = build_nc()
    return _NC_CACHE


def make_in_maps(node, node_mask, punctuation_graph, w_nw, b_nw, w_self, b_self,
                 w_punct):
    node = np.asarray(node, np.float32)
    node_mask = np.asarray(node_mask, np.int32)
    punctuation_graph = np.asarray(punctuation_graph, np.int32)
    w_cat = np.concatenate(
        [np.asarray(w_self, np.float32).T,
         np.asarray(w_punct, np.float32).T,
         np.asarray(w_nw, np.float32).reshape(1, D).T], axis=1)
    bias_ones = np.zeros((1, BO_W), np.float32)
    bias_ones[0, 0:D] = np.asarray(b_self, np.float32).reshape(D)
    bias_ones[0, 2 * D] = np.float32(np.asarray(b_nw).reshape(1)[0])
    bias_ones[0, CAT:] = 1.0
    ident = np.eye(128, dtype=ml_dtypes.bfloat16)
    in_maps = []
    for b in range(NC):
        in_maps.append({
            "punct_T": np.ascontiguousarray(punctuation_graph[b].T),
            "node_T": np.ascontiguousarray(node[b].T),
            "mask_col": np.ascontiguousarray(node_mask[b].reshape(NB, 128).T),
            "w_cat": np.ascontiguousarray(w_cat),
            "bias_ones": bias_ones,
            "ident": ident,
        })
    return in_maps


def unpack_outputs(results):
    x = np.stack([np.asarray(results[i]["out_x"], np.float32) for i in range(NC)])
    all_w = np.zeros((NC, 2, N), np.float32)
    for b in range(NC):
        wr = np.asarray(results[b]["out_w"], np.float32)  # [128, 32]
        for t in range(2):
            all_w[b, t] = wr[:, t * NB : (t + 1) * NB].T.reshape(N)
    return x, all_w


def kernel(node, node_mask, punctuation_graph, w_nw, b_nw, w_self, b_self,
           w_punct, iteration_steps):
    assert int(iteration_steps) == 2
    nc = _get_nc()
    in_maps = make_in_maps(node, node_mask, punctuation_graph, w_nw, b_nw,
                           w_self, b_self, w_punct)
    res = run_bass_kernel_spmd(nc, in_maps, core_ids=list(range(NC)))
    return unpack_outputs(res.results)
